# revision 19
# baseline (speedup 1.0000x reference)
"""AttentionalGNN (SuperGlue-style attention + DGMC spline message passing)
on 8 Trainium2 NeuronCores via the axon PJRT backend.

Sharding: data-parallel over batch B=8 (core b owns image pair b) for
attention / top-k / dense-node DGMC / MLPs. The reference's edge
construction uses per-batch top-k indices WITHOUT batch offsets, so all
batches' edges collide into flat nodes [0, 1024) = (n < 128, all b) —
that graph block is cross-batch entangled. It is computed once per side
(side 0 on device 0, side 1 on device 4), overlapped with the per-batch
dense-node chains on all devices, then broadcast for the final MLPs.

All math follows the reference exactly (fp32, erf-gelu, jax.lax.top_k).
"""

import os

_flags = os.environ.get("NEURON_CC_FLAGS", "")
if "--auto-cast" not in _flags:
    os.environ["NEURON_CC_FLAGS"] = (_flags + " --auto-cast=none").strip()

import numpy as np
import jax
import jax.numpy as jnp
from concurrent.futures import ThreadPoolExecutor

try:
    jax.config.update("jax_compilation_cache_dir", "/tmp/jax_neuron_cache")
    jax.config.update("jax_persistent_cache_min_entry_size_bytes", -1)
    jax.config.update("jax_persistent_cache_min_compile_time_secs", 0.0)
except Exception:
    pass

D_MODEL = 256
NHEAD = 4
HEAD_DIM = D_MODEL // NHEAD
NAMES = ["self", "cross", "self", "cross"]
KNN = 8
KSIZE = 5
GNN_DIM = 64
NUM_STEPS = 10
B = 8
N = 1024
NG = 1024          # graph-block flat nodes (ref flat index n*B+b < 1024)
ND = N - NG // B   # dense nodes per batch per side: n in [128, 1024)


def _layer_norm(x, g, b, eps=1e-5):
    m = x.mean(-1, keepdims=True)
    v = ((x - m) ** 2).mean(-1, keepdims=True)
    return (x - m) * jax.lax.rsqrt(v + eps) * g + b


def _linear(x, p):
    return x @ p["w"] + p["b"]


def _encoder(x_in, src_in, p):
    # x_in, src_in: [C, N] single batch. Returns delta [N, C], awm [N, N].
    x = _layer_norm(x_in.T, p["n0g"], p["n0b"])
    s = _layer_norm(src_in.T, p["n0g"], p["n0b"])
    q = _linear(x, p["q"]).reshape(N, NHEAD, HEAD_DIM)
    k = _linear(s, p["k"]).reshape(N, NHEAD, HEAD_DIM)
    v = _linear(s, p["v"]).reshape(N, NHEAD, HEAD_DIM)
    qk = jnp.einsum("nhd,mhd->nmh", q, k) / jnp.sqrt(jnp.float32(HEAD_DIM))
    A = jax.nn.softmax(qk, axis=1)                     # [N, M, H]
    msg = jnp.einsum("nmh,mhd->nhd", A, v).reshape(N, D_MODEL)
    msg = _layer_norm(_linear(msg, p["merge"]), p["n1g"], p["n1b"])
    msg = _linear(
        jax.nn.gelu(_linear(jnp.concatenate([x, msg], -1), p["mlp1"]),
                    approximate=False),
        p["mlp2"],
    )
    delta = _layer_norm(msg, p["n2g"], p["n2b"])
    awm = A.mean(axis=2)                               # [N, M]
    return delta, awm


def _f_self(desc0_b, desc1_b, lp):
    d0, awm0 = _encoder(desc0_b, desc0_b, lp)
    d1, awm1 = _encoder(desc1_b, desc1_b, lp)
    _, idx0 = jax.lax.top_k(awm0, KNN)
    _, idx1 = jax.lax.top_k(awm1, KNN)
    return d0, d1, idx0, idx1, desc0_b[:, :128], desc1_b[:, :128]


def _f_cross(desc0_b, desc1_b, lp):
    d0, _ = _encoder(desc0_b, desc1_b, lp)
    d1, _ = _encoder(desc1_b, desc0_b, lp)
    return desc0_b + d0.T, desc1_b + d1.T


def _spline_basis(attr):
    pos = attr * (KSIZE - 1)
    lo = jnp.clip(jnp.floor(pos), 0, KSIZE - 2).astype(jnp.int32)
    frac = pos - lo.astype(attr.dtype)
    corners = jnp.array([[0, 0], [0, 1], [1, 0], [1, 1]], jnp.int32)
    kidx = (lo[:, None, 0] + corners[None, :, 0]) * KSIZE + (
        lo[:, None, 1] + corners[None, :, 1])
    w0 = jnp.where(corners[None, :, 0] == 1, frac[:, None, 0], 1 - frac[:, None, 0])
    w1 = jnp.where(corners[None, :, 1] == 1, frac[:, None, 1], 1 - frac[:, None, 1])
    return kidx, w0 * w1


def _graph_conv(h, flat_idx, bw, cp):
    # h: [NG, 64]. flat_idx = src*25 + kidx, [E, 4]. Edge e = (b, n, j) has
    # dst n (ref layout: tile(repeat(arange(N), KNN), B)), so aggregation is
    # a dense reshape-sum — no scatter needed; deg == 64 everywhere.
    hkflat = (h @ cp["W"].transpose(1, 0, 2).reshape(GNN_DIM, -1))  # [NG, 25*64]
    hkflat = hkflat.reshape(NG, KSIZE * KSIZE, GNN_DIM).reshape(-1, GNN_DIM)
    gath = hkflat[flat_idx]                              # [E, 4, 64]
    msg = (bw[:, :, None] * gath).sum(1)                 # [E, 64]
    agg = msg.reshape(B, N, KNN, GNN_DIM).sum(axis=(0, 2)) / 64.0
    return agg + h @ cp["root"] + cp["b"]


def _dense_conv(h, cp):
    return h @ cp["root"] + cp["b"]


def _f_graph(idx_all, dslices, kp, dp):
    # idx_all: [B, N, KNN] i32 (per-batch top-k), dslices: [B, C, 128],
    # kp: [N, 2] (batch-0 keypoints — the reference's global-index quirk).
    # Returns final h over the NG entangled flat nodes.
    src = idx_all.reshape(-1)                            # [B*N*KNN] ref order
    base = jnp.repeat(jnp.arange(N), KNN)
    dst = jnp.tile(base, B)
    attr = kp[src] - kp[dst]
    amax = attr.max(0)
    amin = attr.min(0)
    attr = (attr - amin) / (amax - amin)
    kidx, bw = _spline_basis(attr)

    flat_idx = src[:, None] * (KSIZE * KSIZE) + kidx     # [E, 4]

    xg = jnp.transpose(dslices, (2, 0, 1)).reshape(NG, D_MODEL)  # row n*B+b
    h = jax.nn.relu(_linear(xg, dp["proj"]))

    def step(h, _):
        h1 = jax.nn.relu(_graph_conv(h, flat_idx, bw, dp["conv1"]))
        h2 = jax.nn.relu(_graph_conv(h1, flat_idx, bw, dp["conv2"]))
        h = h + _linear(jnp.concatenate([h, h1, h2], -1), dp["lin"])
        return h, None
    h, _ = jax.lax.scan(step, h, None, length=NUM_STEPS)
    return h


def _f_dense(desc0_b, desc1_b, dp):
    # Dense (non-entangled) flat nodes for this batch: n in [128, N).
    h = jax.nn.relu(_linear(
        jnp.concatenate([desc0_b[:, 128:], desc1_b[:, 128:]], 1).T,
        dp["proj"]))                                         # [2*896, 64]

    def step(h, _):
        h1 = jax.nn.relu(_dense_conv(h, dp["conv1"]))
        h2 = jax.nn.relu(_dense_conv(h1, dp["conv2"]))
        h = h + _linear(jnp.concatenate([h, h1, h2], -1), dp["lin"])
        return h, None
    h, _ = jax.lax.scan(step, h, None, length=NUM_STEPS)
    return h[:ND], h[ND:]


def _f_finish(delta0, delta1, hg0, hg1, hd0, hd1, desc0_b, desc1_b, b_idx, params):
    # Assemble d = [B,64,N] row for this batch: n<128 from the graph block
    # (flat n*B+b), n>=128 from the local dense chain.
    outs = []
    for delta, hg, hd, desc in ((delta0, hg0, hd0, desc0_b),
                                (delta1, hg1, hd1, desc1_b)):
        dg = jax.lax.dynamic_index_in_dim(
            hg.reshape(128, B, GNN_DIM), b_idx, axis=1, keepdims=False)
        d = jnp.concatenate([dg, hd], 0)                     # [N, 64]
        d = _layer_norm(d, params["n1g"], params["n1b"])
        cat = jnp.concatenate([delta, d], -1)                # [N, 320]
        m = _linear(jax.nn.gelu(_linear(cat, params["m1"]), approximate=False),
                    params["m2"])
        outs.append(desc + m.T)
    return outs[0], outs[1]


def _np_graph(idx_all, dslices, kp, dp):
    # Host fallback for the entangled graph block (exact same math, numpy).
    src = idx_all.reshape(-1).astype(np.int64)
    base = np.repeat(np.arange(N), KNN)
    dst = np.tile(base, B)
    attr = kp[src] - kp[dst]
    amax = attr.max(0)
    amin = attr.min(0)
    attr = (attr - amin) / (amax - amin)
    pos = attr * (KSIZE - 1)
    lo = np.clip(np.floor(pos), 0, KSIZE - 2).astype(np.int64)
    frac = (pos - lo).astype(np.float32)
    corners = np.array([[0, 0], [0, 1], [1, 0], [1, 1]], np.int64)
    kidx = (lo[:, None, 0] + corners[None, :, 0]) * KSIZE + (
        lo[:, None, 1] + corners[None, :, 1])
    w0 = np.where(corners[None, :, 0] == 1, frac[:, None, 0], 1 - frac[:, None, 0])
    w1 = np.where(corners[None, :, 1] == 1, frac[:, None, 1], 1 - frac[:, None, 1])
    bw = (w0 * w1).astype(np.float32)
    flat_idx = src[:, None] * (KSIZE * KSIZE) + kidx

    xg = np.transpose(dslices, (2, 0, 1)).reshape(NG, D_MODEL)
    h = np.maximum(xg @ dp["proj"]["w"] + dp["proj"]["b"], 0.0)
    w1f = dp["conv1"]["W"].transpose(1, 0, 2).reshape(GNN_DIM, -1)
    w2f = dp["conv2"]["W"].transpose(1, 0, 2).reshape(GNN_DIM, -1)

    def conv(x, wf, cp):
        hkflat = (x @ wf).reshape(-1, GNN_DIM)
        msg = (bw[:, :, None] * hkflat[flat_idx]).sum(1)
        agg = msg.reshape(B, N, KNN, GNN_DIM).sum(axis=(0, 2)) / 64.0
        return agg + x @ cp["root"] + cp["b"]

    for _ in range(NUM_STEPS):
        h1 = np.maximum(conv(h, w1f, dp["conv1"]), 0.0)
        h2 = np.maximum(conv(h1, w2f, dp["conv2"]), 0.0)
        h = h + np.concatenate([h, h1, h2], -1) @ dp["lin"]["w"] + dp["lin"]["b"]
    return h


# neuronx-cc densifies the [65536,4]-index gather into a petabyte-scale
# one-hot (NCC_EXSP001) with vector_dynamic_offsets DGE disabled, so the
# entangled graph block runs on host (exact same math) overlapped with the
# on-device dense-node chains.
_USE_NP_GRAPH = os.environ.get("ATTGNN_NP_GRAPH", "1") == "1"

_jit_self = jax.jit(_f_self)
_jit_cross = jax.jit(_f_cross)
_jit_graph = jax.jit(_f_graph)
_jit_dense = jax.jit(_f_dense)
_jit_finish = jax.jit(_f_finish)


def kernel(desc0, desc1, kpts0, kpts1, params):
    import time as _time
    _t = {"self": 0.0, "pull": 0.0, "graph": 0.0, "dense_wait": 0.0,
          "finish": 0.0, "cross": 0.0}
    _tick = _time.time
    devs = jax.devices()[:B]
    g0_dev, g1_dev = devs[0], devs[4]

    params = jax.tree.map(np.asarray, params)
    put = jax.device_put

    # Replicate parameters (small) to every device once.
    lp_dev = [[put(jax.tree.map(jnp.asarray, lp), d) for lp in params["layers"]]
              for d in devs]
    dgmc_dev = [put({k: params["dgmc"][k] for k in params["dgmc"]}, d) for d in devs]
    fin_keys = {"m1": params["m1"], "m2": params["m2"],
                "n1g": params["n1g"], "n1b": params["n1b"]}
    fin_dev = [put(fin_keys, d) for d in devs]
    kp0_g = put(np.asarray(kpts0[0]), g0_dev)
    kp1_g = put(np.asarray(kpts1[0]), g1_dev)
    _np_kp0 = np.asarray(kpts0[0], dtype=np.float32)
    _np_kp1 = np.asarray(kpts1[0], dtype=np.float32)
    _np_dgmc = params["dgmc"]

    d0 = [put(np.asarray(desc0[b]), devs[b]) for b in range(B)]
    d1 = [put(np.asarray(desc1[b]), devs[b]) for b in range(B)]

    for li, name in enumerate(NAMES):
        if name == "cross":
            t0 = _tick()
            res = [_jit_cross(d0[b], d1[b], lp_dev[b][li]) for b in range(B)]
            d0 = [r[0] for r in res]
            d1 = [r[1] for r in res]
            jax.block_until_ready(d0)
            _t["cross"] += _tick() - t0
        else:
            t0 = _tick()
            res = [_jit_self(d0[b], d1[b], lp_dev[b][li]) for b in range(B)]
            # Pull the small idx / desc-slice outputs to host, assemble the
            # entangled graph-block inputs, dispatch side 0 -> dev0 and
            # side 1 -> dev4 while every device runs its dense chains.
            # Dispatch the per-batch dense chains first (async on all 8
            # devices), then run/issue the two entangled graph blocks.
            dense = [_jit_dense(d0[b], d1[b], dgmc_dev[b]) for b in range(B)]
            _t["self"] += _tick() - t0
            t0 = _tick()
            idx0 = np.stack([np.asarray(r[2]) for r in res])
            idx1 = np.stack([np.asarray(r[3]) for r in res])
            ds0 = np.stack([np.asarray(r[4]) for r in res])
            ds1 = np.stack([np.asarray(r[5]) for r in res])
            _t["pull"] += _tick() - t0
            t0 = _tick()
            if _USE_NP_GRAPH:
                with ThreadPoolExecutor(2) as ex:
                    f0 = ex.submit(_np_graph, idx0, ds0, _np_kp0, _np_dgmc)
                    f1 = ex.submit(_np_graph, idx1, ds1, _np_kp1, _np_dgmc)
                    hg0_np = f0.result()
                    hg1_np = f1.result()
            else:
                hg0 = _jit_graph(put(idx0, g0_dev), put(ds0, g0_dev), kp0_g,
                                 dgmc_dev[0])
                hg1 = _jit_graph(put(idx1, g1_dev), put(ds1, g1_dev), kp1_g,
                                 dgmc_dev[4])
                hg0_np = np.asarray(hg0)
                hg1_np = np.asarray(hg1)
            _t["graph"] += _tick() - t0
            t0 = _tick()
            jax.block_until_ready(dense)
            _t["dense_wait"] += _tick() - t0
            t0 = _tick()
            new0, new1 = [], []
            for b in range(B):
                o = _jit_finish(res[b][0], res[b][1],
                                put(hg0_np, devs[b]), put(hg1_np, devs[b]),
                                dense[b][0], dense[b][1],
                                d0[b], d1[b],
                                put(np.int32(b), devs[b]), fin_dev[b])
                new0.append(o[0])
                new1.append(o[1])
            d0, d1 = new0, new1
            jax.block_until_ready(d0)
            _t["finish"] += _tick() - t0

    out0 = np.stack([np.asarray(x) for x in d0])
    out1 = np.stack([np.asarray(x) for x in d1])
    if os.environ.get("ATTGNN_TIMING", "0") == "1":
        print("phase times:", {k: round(v, 2) for k, v in _t.items()})
    return out0, out1


# revision 21
# speedup vs baseline: 1.3653x; 1.3653x over previous
"""AttentionalGNN (SuperGlue-style attention + DGMC spline message passing)
on 8 Trainium2 NeuronCores via the axon PJRT backend.

Sharding: data-parallel over batch B=8 (core b owns image pair b) for
attention / top-k / dense-node DGMC / MLPs. The reference's edge
construction uses per-batch top-k indices WITHOUT batch offsets, so all
batches' edges collide into flat nodes [0, 1024) = (n < 128, all b) —
that graph block is cross-batch entangled. It is computed once per side
(side 0 on device 0, side 1 on device 4), overlapped with the per-batch
dense-node chains on all devices, then broadcast for the final MLPs.

All math follows the reference exactly (fp32, erf-gelu, jax.lax.top_k).
"""

import os

_flags = os.environ.get("NEURON_CC_FLAGS", "")
if "--auto-cast" not in _flags:
    os.environ["NEURON_CC_FLAGS"] = (_flags + " --auto-cast=none").strip()

import numpy as np
import jax
import jax.numpy as jnp
from concurrent.futures import ThreadPoolExecutor

try:
    jax.config.update("jax_compilation_cache_dir", "/tmp/jax_neuron_cache")
    jax.config.update("jax_persistent_cache_min_entry_size_bytes", -1)
    jax.config.update("jax_persistent_cache_min_compile_time_secs", 0.0)
except Exception:
    pass

D_MODEL = 256
NHEAD = 4
HEAD_DIM = D_MODEL // NHEAD
NAMES = ["self", "cross", "self", "cross"]
KNN = 8
KSIZE = 5
GNN_DIM = 64
NUM_STEPS = 10
B = 8
N = 1024
NG = 1024          # graph-block flat nodes (ref flat index n*B+b < 1024)
ND = N - NG // B   # dense nodes per batch per side: n in [128, 1024)


def _layer_norm(x, g, b, eps=1e-5):
    m = x.mean(-1, keepdims=True)
    v = ((x - m) ** 2).mean(-1, keepdims=True)
    return (x - m) * jax.lax.rsqrt(v + eps) * g + b


def _linear(x, p):
    return x @ p["w"] + p["b"]


def _encoder(x_in, src_in, p):
    # x_in, src_in: [C, N] single batch. Returns delta [N, C], awm [N, N].
    x = _layer_norm(x_in.T, p["n0g"], p["n0b"])
    s = _layer_norm(src_in.T, p["n0g"], p["n0b"])
    q = _linear(x, p["q"]).reshape(N, NHEAD, HEAD_DIM)
    k = _linear(s, p["k"]).reshape(N, NHEAD, HEAD_DIM)
    v = _linear(s, p["v"]).reshape(N, NHEAD, HEAD_DIM)
    qk = jnp.einsum("nhd,mhd->nmh", q, k) / jnp.sqrt(jnp.float32(HEAD_DIM))
    A = jax.nn.softmax(qk, axis=1)                     # [N, M, H]
    msg = jnp.einsum("nmh,mhd->nhd", A, v).reshape(N, D_MODEL)
    msg = _layer_norm(_linear(msg, p["merge"]), p["n1g"], p["n1b"])
    msg = _linear(
        jax.nn.gelu(_linear(jnp.concatenate([x, msg], -1), p["mlp1"]),
                    approximate=False),
        p["mlp2"],
    )
    delta = _layer_norm(msg, p["n2g"], p["n2b"])
    awm = A.mean(axis=2)                               # [N, M]
    return delta, awm


def _f_self(desc0_b, desc1_b, lp):
    d0, awm0 = _encoder(desc0_b, desc0_b, lp)
    d1, awm1 = _encoder(desc1_b, desc1_b, lp)
    _, idx0 = jax.lax.top_k(awm0, KNN)
    _, idx1 = jax.lax.top_k(awm1, KNN)
    return d0, d1, idx0, idx1, desc0_b[:, :128], desc1_b[:, :128]


def _f_cross(desc0_b, desc1_b, lp):
    d0, _ = _encoder(desc0_b, desc1_b, lp)
    d1, _ = _encoder(desc1_b, desc0_b, lp)
    return desc0_b + d0.T, desc1_b + d1.T


def _spline_basis(attr):
    pos = attr * (KSIZE - 1)
    lo = jnp.clip(jnp.floor(pos), 0, KSIZE - 2).astype(jnp.int32)
    frac = pos - lo.astype(attr.dtype)
    corners = jnp.array([[0, 0], [0, 1], [1, 0], [1, 1]], jnp.int32)
    kidx = (lo[:, None, 0] + corners[None, :, 0]) * KSIZE + (
        lo[:, None, 1] + corners[None, :, 1])
    w0 = jnp.where(corners[None, :, 0] == 1, frac[:, None, 0], 1 - frac[:, None, 0])
    w1 = jnp.where(corners[None, :, 1] == 1, frac[:, None, 1], 1 - frac[:, None, 1])
    return kidx, w0 * w1


def _graph_conv(h, flat_idx, bw, cp):
    # h: [NG, 64]. flat_idx = src*25 + kidx, [E, 4]. Edge e = (b, n, j) has
    # dst n (ref layout: tile(repeat(arange(N), KNN), B)), so aggregation is
    # a dense reshape-sum — no scatter needed; deg == 64 everywhere.
    hkflat = (h @ cp["W"].transpose(1, 0, 2).reshape(GNN_DIM, -1))  # [NG, 25*64]
    hkflat = hkflat.reshape(NG, KSIZE * KSIZE, GNN_DIM).reshape(-1, GNN_DIM)
    gath = hkflat[flat_idx]                              # [E, 4, 64]
    msg = (bw[:, :, None] * gath).sum(1)                 # [E, 64]
    agg = msg.reshape(B, N, KNN, GNN_DIM).sum(axis=(0, 2)) / 64.0
    return agg + h @ cp["root"] + cp["b"]


def _dense_conv(h, cp):
    return h @ cp["root"] + cp["b"]


def _f_graph(idx_all, dslices, kp, dp):
    # idx_all: [B, N, KNN] i32 (per-batch top-k), dslices: [B, C, 128],
    # kp: [N, 2] (batch-0 keypoints — the reference's global-index quirk).
    # Returns final h over the NG entangled flat nodes.
    src = idx_all.reshape(-1)                            # [B*N*KNN] ref order
    base = jnp.repeat(jnp.arange(N), KNN)
    dst = jnp.tile(base, B)
    attr = kp[src] - kp[dst]
    amax = attr.max(0)
    amin = attr.min(0)
    attr = (attr - amin) / (amax - amin)
    kidx, bw = _spline_basis(attr)

    flat_idx = src[:, None] * (KSIZE * KSIZE) + kidx     # [E, 4]

    xg = jnp.transpose(dslices, (2, 0, 1)).reshape(NG, D_MODEL)  # row n*B+b
    h = jax.nn.relu(_linear(xg, dp["proj"]))

    def step(h, _):
        h1 = jax.nn.relu(_graph_conv(h, flat_idx, bw, dp["conv1"]))
        h2 = jax.nn.relu(_graph_conv(h1, flat_idx, bw, dp["conv2"]))
        h = h + _linear(jnp.concatenate([h, h1, h2], -1), dp["lin"])
        return h, None
    h, _ = jax.lax.scan(step, h, None, length=NUM_STEPS)
    return h


def _f_dense(desc0_b, desc1_b, dp):
    # Dense (non-entangled) flat nodes for this batch: n in [128, N).
    h = jax.nn.relu(_linear(
        jnp.concatenate([desc0_b[:, 128:], desc1_b[:, 128:]], 1).T,
        dp["proj"]))                                         # [2*896, 64]

    def step(h, _):
        h1 = jax.nn.relu(_dense_conv(h, dp["conv1"]))
        h2 = jax.nn.relu(_dense_conv(h1, dp["conv2"]))
        h = h + _linear(jnp.concatenate([h, h1, h2], -1), dp["lin"])
        return h, None
    h, _ = jax.lax.scan(step, h, None, length=NUM_STEPS)
    return h[:ND], h[ND:]


def _f_finish(delta0, delta1, hg0, hg1, hd0, hd1, desc0_b, desc1_b, b_idx, params):
    # Assemble d = [B,64,N] row for this batch: n<128 from the graph block
    # (flat n*B+b), n>=128 from the local dense chain.
    outs = []
    for delta, hg, hd, desc in ((delta0, hg0, hd0, desc0_b),
                                (delta1, hg1, hd1, desc1_b)):
        dg = jax.lax.dynamic_index_in_dim(
            hg.reshape(128, B, GNN_DIM), b_idx, axis=1, keepdims=False)
        d = jnp.concatenate([dg, hd], 0)                     # [N, 64]
        d = _layer_norm(d, params["n1g"], params["n1b"])
        cat = jnp.concatenate([delta, d], -1)                # [N, 320]
        m = _linear(jax.nn.gelu(_linear(cat, params["m1"]), approximate=False),
                    params["m2"])
        outs.append(desc + m.T)
    return outs[0], outs[1]


def _np_graph(idx_all, dslices, kp, dp):
    # Host fallback for the entangled graph block (exact same math, numpy).
    src = idx_all.reshape(-1).astype(np.int64)
    base = np.repeat(np.arange(N), KNN)
    dst = np.tile(base, B)
    attr = kp[src] - kp[dst]
    amax = attr.max(0)
    amin = attr.min(0)
    attr = (attr - amin) / (amax - amin)
    pos = attr * (KSIZE - 1)
    lo = np.clip(np.floor(pos), 0, KSIZE - 2).astype(np.int64)
    frac = (pos - lo).astype(np.float32)
    corners = np.array([[0, 0], [0, 1], [1, 0], [1, 1]], np.int64)
    kidx = (lo[:, None, 0] + corners[None, :, 0]) * KSIZE + (
        lo[:, None, 1] + corners[None, :, 1])
    w0 = np.where(corners[None, :, 0] == 1, frac[:, None, 0], 1 - frac[:, None, 0])
    w1 = np.where(corners[None, :, 1] == 1, frac[:, None, 1], 1 - frac[:, None, 1])
    bw = (w0 * w1).astype(np.float32)
    flat_idx = src[:, None] * (KSIZE * KSIZE) + kidx

    xg = np.transpose(dslices, (2, 0, 1)).reshape(NG, D_MODEL)
    h = np.maximum(xg @ dp["proj"]["w"] + dp["proj"]["b"], 0.0)
    w1f = dp["conv1"]["W"].transpose(1, 0, 2).reshape(GNN_DIM, -1)
    w2f = dp["conv2"]["W"].transpose(1, 0, 2).reshape(GNN_DIM, -1)

    bw_c = [np.ascontiguousarray(bw[:, c]) for c in range(4)]
    fi_c = [np.ascontiguousarray(flat_idx[:, c]) for c in range(4)]

    def conv(x, wf, cp):
        hkflat = (x @ wf).reshape(-1, GNN_DIM)
        msg = np.take(hkflat, fi_c[0], axis=0) * bw_c[0][:, None]
        for c in range(1, 4):
            msg += np.take(hkflat, fi_c[c], axis=0) * bw_c[c][:, None]
        agg = msg.reshape(B, N * KNN * GNN_DIM).sum(0)
        agg = agg.reshape(N, KNN, GNN_DIM).sum(1) * (1.0 / 64.0)
        return agg + x @ cp["root"] + cp["b"]

    for _ in range(NUM_STEPS):
        h1 = np.maximum(conv(h, w1f, dp["conv1"]), 0.0)
        h2 = np.maximum(conv(h1, w2f, dp["conv2"]), 0.0)
        h = h + np.concatenate([h, h1, h2], -1) @ dp["lin"]["w"] + dp["lin"]["b"]
    return h


# neuronx-cc densifies the [65536,4]-index gather into a petabyte-scale
# one-hot (NCC_EXSP001) with vector_dynamic_offsets DGE disabled, so the
# entangled graph block runs on host (exact same math) overlapped with the
# on-device dense-node chains.
_USE_NP_GRAPH = os.environ.get("ATTGNN_NP_GRAPH", "1") == "1"

_jit_self = jax.jit(_f_self)
_jit_cross = jax.jit(_f_cross)
_jit_graph = jax.jit(_f_graph)
_jit_dense = jax.jit(_f_dense)
_jit_finish = jax.jit(_f_finish)


def kernel(desc0, desc1, kpts0, kpts1, params):
    import time as _time
    _t = {"self": 0.0, "pull": 0.0, "graph": 0.0, "dense_wait": 0.0,
          "finish": 0.0, "cross": 0.0}
    _tick = _time.time
    devs = jax.devices()[:B]
    g0_dev, g1_dev = devs[0], devs[4]

    params = jax.tree.map(np.asarray, params)
    put = jax.device_put

    # Replicate parameters (small) to every device once.
    lp_dev = [[put(jax.tree.map(jnp.asarray, lp), d) for lp in params["layers"]]
              for d in devs]
    dgmc_dev = [put({k: params["dgmc"][k] for k in params["dgmc"]}, d) for d in devs]
    fin_keys = {"m1": params["m1"], "m2": params["m2"],
                "n1g": params["n1g"], "n1b": params["n1b"]}
    fin_dev = [put(fin_keys, d) for d in devs]
    kp0_g = put(np.asarray(kpts0[0]), g0_dev)
    kp1_g = put(np.asarray(kpts1[0]), g1_dev)
    _np_kp0 = np.asarray(kpts0[0], dtype=np.float32)
    _np_kp1 = np.asarray(kpts1[0], dtype=np.float32)
    _np_dgmc = params["dgmc"]

    d0 = [put(np.asarray(desc0[b]), devs[b]) for b in range(B)]
    d1 = [put(np.asarray(desc1[b]), devs[b]) for b in range(B)]

    for li, name in enumerate(NAMES):
        if name == "cross":
            t0 = _tick()
            res = [_jit_cross(d0[b], d1[b], lp_dev[b][li]) for b in range(B)]
            d0 = [r[0] for r in res]
            d1 = [r[1] for r in res]
            jax.block_until_ready(d0)
            _t["cross"] += _tick() - t0
        else:
            t0 = _tick()
            res = [_jit_self(d0[b], d1[b], lp_dev[b][li]) for b in range(B)]
            # Pull the small idx / desc-slice outputs to host, assemble the
            # entangled graph-block inputs, dispatch side 0 -> dev0 and
            # side 1 -> dev4 while every device runs its dense chains.
            # Dispatch the per-batch dense chains first (async on all 8
            # devices), then run/issue the two entangled graph blocks.
            dense = [_jit_dense(d0[b], d1[b], dgmc_dev[b]) for b in range(B)]
            _t["self"] += _tick() - t0
            t0 = _tick()
            # 32 small device->host pulls; each is a tunnel roundtrip, so
            # overlap them across a thread pool.
            with ThreadPoolExecutor(16) as ex:
                flat = list(ex.map(np.asarray,
                                   [r[i] for r in res for i in (2, 3, 4, 5)]))
            idx0 = np.stack(flat[0::4])
            idx1 = np.stack(flat[1::4])
            ds0 = np.stack(flat[2::4])
            ds1 = np.stack(flat[3::4])
            _t["pull"] += _tick() - t0
            t0 = _tick()
            if _USE_NP_GRAPH:
                with ThreadPoolExecutor(2) as ex:
                    f0 = ex.submit(_np_graph, idx0, ds0, _np_kp0, _np_dgmc)
                    f1 = ex.submit(_np_graph, idx1, ds1, _np_kp1, _np_dgmc)
                    hg0_np = f0.result()
                    hg1_np = f1.result()
            else:
                hg0 = _jit_graph(put(idx0, g0_dev), put(ds0, g0_dev), kp0_g,
                                 dgmc_dev[0])
                hg1 = _jit_graph(put(idx1, g1_dev), put(ds1, g1_dev), kp1_g,
                                 dgmc_dev[4])
                hg0_np = np.asarray(hg0)
                hg1_np = np.asarray(hg1)
            _t["graph"] += _tick() - t0
            t0 = _tick()
            jax.block_until_ready(dense)
            _t["dense_wait"] += _tick() - t0
            t0 = _tick()
            new0, new1 = [], []
            for b in range(B):
                o = _jit_finish(res[b][0], res[b][1],
                                put(hg0_np, devs[b]), put(hg1_np, devs[b]),
                                dense[b][0], dense[b][1],
                                d0[b], d1[b],
                                put(np.int32(b), devs[b]), fin_dev[b])
                new0.append(o[0])
                new1.append(o[1])
            d0, d1 = new0, new1
            jax.block_until_ready(d0)
            _t["finish"] += _tick() - t0

    out0 = np.stack([np.asarray(x) for x in d0])
    out1 = np.stack([np.asarray(x) for x in d1])
    if os.environ.get("ATTGNN_TIMING", "0") == "1":
        print("phase times:", {k: round(v, 2) for k, v in _t.items()})
    return out0, out1


# revision 22
# speedup vs baseline: 1.6289x; 1.1931x over previous
"""AttentionalGNN (SuperGlue-style attention + DGMC spline message passing)
on 8 Trainium2 NeuronCores via the axon PJRT backend.

Sharding: data-parallel over batch B=8 (core b owns image pair b) for
attention / top-k / dense-node DGMC / MLPs. The reference's edge
construction uses per-batch top-k indices WITHOUT batch offsets, so all
batches' edges collide into flat nodes [0, 1024) = (n < 128, all b) —
that graph block is cross-batch entangled. It is computed once per side
(side 0 on device 0, side 1 on device 4), overlapped with the per-batch
dense-node chains on all devices, then broadcast for the final MLPs.

All math follows the reference exactly (fp32, erf-gelu, jax.lax.top_k).
"""

import os

_flags = os.environ.get("NEURON_CC_FLAGS", "")
if "--auto-cast" not in _flags:
    os.environ["NEURON_CC_FLAGS"] = (_flags + " --auto-cast=none").strip()

import numpy as np
import jax
import jax.numpy as jnp
from concurrent.futures import ThreadPoolExecutor

try:
    jax.config.update("jax_compilation_cache_dir", "/tmp/jax_neuron_cache")
    jax.config.update("jax_persistent_cache_min_entry_size_bytes", -1)
    jax.config.update("jax_persistent_cache_min_compile_time_secs", 0.0)
except Exception:
    pass

D_MODEL = 256
NHEAD = 4
HEAD_DIM = D_MODEL // NHEAD
NAMES = ["self", "cross", "self", "cross"]
KNN = 8
KSIZE = 5
GNN_DIM = 64
NUM_STEPS = 10
B = 8
N = 1024
NG = 1024          # graph-block flat nodes (ref flat index n*B+b < 1024)
ND = N - NG // B   # dense nodes per batch per side: n in [128, 1024)


def _layer_norm(x, g, b, eps=1e-5):
    m = x.mean(-1, keepdims=True)
    v = ((x - m) ** 2).mean(-1, keepdims=True)
    return (x - m) * jax.lax.rsqrt(v + eps) * g + b


def _linear(x, p):
    return x @ p["w"] + p["b"]


def _encoder(x_in, src_in, p):
    # x_in, src_in: [C, N] single batch. Returns delta [N, C], awm [N, N].
    x = _layer_norm(x_in.T, p["n0g"], p["n0b"])
    s = _layer_norm(src_in.T, p["n0g"], p["n0b"])
    q = _linear(x, p["q"]).reshape(N, NHEAD, HEAD_DIM)
    k = _linear(s, p["k"]).reshape(N, NHEAD, HEAD_DIM)
    v = _linear(s, p["v"]).reshape(N, NHEAD, HEAD_DIM)
    qk = jnp.einsum("nhd,mhd->nmh", q, k) / jnp.sqrt(jnp.float32(HEAD_DIM))
    A = jax.nn.softmax(qk, axis=1)                     # [N, M, H]
    msg = jnp.einsum("nmh,mhd->nhd", A, v).reshape(N, D_MODEL)
    msg = _layer_norm(_linear(msg, p["merge"]), p["n1g"], p["n1b"])
    msg = _linear(
        jax.nn.gelu(_linear(jnp.concatenate([x, msg], -1), p["mlp1"]),
                    approximate=False),
        p["mlp2"],
    )
    delta = _layer_norm(msg, p["n2g"], p["n2b"])
    awm = A.mean(axis=2)                               # [N, M]
    return delta, awm


def _f_self(desc0_b, desc1_b, lp):
    d0, awm0 = _encoder(desc0_b, desc0_b, lp)
    d1, awm1 = _encoder(desc1_b, desc1_b, lp)
    _, idx0 = jax.lax.top_k(awm0, KNN)
    _, idx1 = jax.lax.top_k(awm1, KNN)
    return d0, d1, idx0, idx1, desc0_b[:, :128], desc1_b[:, :128]


def _f_cross(desc0_b, desc1_b, lp):
    d0, _ = _encoder(desc0_b, desc1_b, lp)
    d1, _ = _encoder(desc1_b, desc0_b, lp)
    return desc0_b + d0.T, desc1_b + d1.T


def _spline_basis(attr):
    pos = attr * (KSIZE - 1)
    lo = jnp.clip(jnp.floor(pos), 0, KSIZE - 2).astype(jnp.int32)
    frac = pos - lo.astype(attr.dtype)
    corners = jnp.array([[0, 0], [0, 1], [1, 0], [1, 1]], jnp.int32)
    kidx = (lo[:, None, 0] + corners[None, :, 0]) * KSIZE + (
        lo[:, None, 1] + corners[None, :, 1])
    w0 = jnp.where(corners[None, :, 0] == 1, frac[:, None, 0], 1 - frac[:, None, 0])
    w1 = jnp.where(corners[None, :, 1] == 1, frac[:, None, 1], 1 - frac[:, None, 1])
    return kidx, w0 * w1


def _graph_conv(h, flat_idx, bw, cp):
    # h: [NG, 64]. flat_idx = src*25 + kidx, [E, 4]. Edge e = (b, n, j) has
    # dst n (ref layout: tile(repeat(arange(N), KNN), B)), so aggregation is
    # a dense reshape-sum — no scatter needed; deg == 64 everywhere.
    hkflat = (h @ cp["W"].transpose(1, 0, 2).reshape(GNN_DIM, -1))  # [NG, 25*64]
    hkflat = hkflat.reshape(NG, KSIZE * KSIZE, GNN_DIM).reshape(-1, GNN_DIM)
    gath = hkflat[flat_idx]                              # [E, 4, 64]
    msg = (bw[:, :, None] * gath).sum(1)                 # [E, 64]
    agg = msg.reshape(B, N, KNN, GNN_DIM).sum(axis=(0, 2)) / 64.0
    return agg + h @ cp["root"] + cp["b"]


def _dense_conv(h, cp):
    return h @ cp["root"] + cp["b"]


def _f_graph(idx_all, dslices, kp, dp):
    # idx_all: [B, N, KNN] i32 (per-batch top-k), dslices: [B, C, 128],
    # kp: [N, 2] (batch-0 keypoints — the reference's global-index quirk).
    # Returns final h over the NG entangled flat nodes.
    src = idx_all.reshape(-1)                            # [B*N*KNN] ref order
    base = jnp.repeat(jnp.arange(N), KNN)
    dst = jnp.tile(base, B)
    attr = kp[src] - kp[dst]
    amax = attr.max(0)
    amin = attr.min(0)
    attr = (attr - amin) / (amax - amin)
    kidx, bw = _spline_basis(attr)

    flat_idx = src[:, None] * (KSIZE * KSIZE) + kidx     # [E, 4]

    xg = jnp.transpose(dslices, (2, 0, 1)).reshape(NG, D_MODEL)  # row n*B+b
    h = jax.nn.relu(_linear(xg, dp["proj"]))

    def step(h, _):
        h1 = jax.nn.relu(_graph_conv(h, flat_idx, bw, dp["conv1"]))
        h2 = jax.nn.relu(_graph_conv(h1, flat_idx, bw, dp["conv2"]))
        h = h + _linear(jnp.concatenate([h, h1, h2], -1), dp["lin"])
        return h, None
    h, _ = jax.lax.scan(step, h, None, length=NUM_STEPS)
    return h


def _f_dense(desc0_b, desc1_b, dp):
    # Dense (non-entangled) flat nodes for this batch: n in [128, N).
    h = jax.nn.relu(_linear(
        jnp.concatenate([desc0_b[:, 128:], desc1_b[:, 128:]], 1).T,
        dp["proj"]))                                         # [2*896, 64]

    def step(h, _):
        h1 = jax.nn.relu(_dense_conv(h, dp["conv1"]))
        h2 = jax.nn.relu(_dense_conv(h1, dp["conv2"]))
        h = h + _linear(jnp.concatenate([h, h1, h2], -1), dp["lin"])
        return h, None
    h, _ = jax.lax.scan(step, h, None, length=NUM_STEPS)
    return h[:ND], h[ND:]


def _f_finish(delta0, delta1, hg0, hg1, hd0, hd1, desc0_b, desc1_b, b_idx, params):
    # Assemble d = [B,64,N] row for this batch: n<128 from the graph block
    # (flat n*B+b), n>=128 from the local dense chain.
    outs = []
    for delta, hg, hd, desc in ((delta0, hg0, hd0, desc0_b),
                                (delta1, hg1, hd1, desc1_b)):
        dg = jax.lax.dynamic_index_in_dim(
            hg.reshape(128, B, GNN_DIM), b_idx, axis=1, keepdims=False)
        d = jnp.concatenate([dg, hd], 0)                     # [N, 64]
        d = _layer_norm(d, params["n1g"], params["n1b"])
        cat = jnp.concatenate([delta, d], -1)                # [N, 320]
        m = _linear(jax.nn.gelu(_linear(cat, params["m1"]), approximate=False),
                    params["m2"])
        outs.append(desc + m.T)
    return outs[0], outs[1]


def _np_graph(idx_all, dslices, kp, dp):
    # Host fallback for the entangled graph block (exact same math, numpy).
    src = idx_all.reshape(-1).astype(np.int64)
    base = np.repeat(np.arange(N), KNN)
    dst = np.tile(base, B)
    attr = kp[src] - kp[dst]
    amax = attr.max(0)
    amin = attr.min(0)
    attr = (attr - amin) / (amax - amin)
    pos = attr * (KSIZE - 1)
    lo = np.clip(np.floor(pos), 0, KSIZE - 2).astype(np.int64)
    frac = (pos - lo).astype(np.float32)
    corners = np.array([[0, 0], [0, 1], [1, 0], [1, 1]], np.int64)
    kidx = (lo[:, None, 0] + corners[None, :, 0]) * KSIZE + (
        lo[:, None, 1] + corners[None, :, 1])
    w0 = np.where(corners[None, :, 0] == 1, frac[:, None, 0], 1 - frac[:, None, 0])
    w1 = np.where(corners[None, :, 1] == 1, frac[:, None, 1], 1 - frac[:, None, 1])
    bw = (w0 * w1).astype(np.float32)
    # Exact algebraic reorganization: agg[n] = (1/64) sum_k (A_k @ h) @ W_k
    # with A_k[n, m] = sum of bw over edge-corners (src=m, dst=n, cell=k).
    # A is fixed across all 20 convs of this layer-side; build it once with
    # bincount (C-speed) and turn every conv into pure BLAS — no gathers.
    KT = KSIZE * KSIZE
    idxA = (kidx.astype(np.int64) * (N * N)
            + dst[:, None].astype(np.int64) * N
            + src[:, None])                                  # [E, 4]
    A = np.bincount(idxA.ravel(), weights=bw.ravel().astype(np.float64),
                    minlength=KT * N * N)
    A2d = np.ascontiguousarray(A.reshape(KT * N, N).astype(np.float32))

    xg = np.transpose(dslices, (2, 0, 1)).reshape(NG, D_MODEL)
    h = np.maximum(xg @ dp["proj"]["w"] + dp["proj"]["b"], 0.0)

    def conv(x, cp):
        Y = (A2d @ x).reshape(KT, N, GNN_DIM)                # [25, N, 64]
        agg = Y[0] @ cp["W"][0]
        for k in range(1, KT):
            agg += Y[k] @ cp["W"][k]
        agg *= (1.0 / 64.0)
        return agg + x @ cp["root"] + cp["b"]

    for _ in range(NUM_STEPS):
        h1 = np.maximum(conv(h, dp["conv1"]), 0.0)
        h2 = np.maximum(conv(h1, dp["conv2"]), 0.0)
        h = h + np.concatenate([h, h1, h2], -1) @ dp["lin"]["w"] + dp["lin"]["b"]
    return h


# neuronx-cc densifies the [65536,4]-index gather into a petabyte-scale
# one-hot (NCC_EXSP001) with vector_dynamic_offsets DGE disabled, so the
# entangled graph block runs on host (exact same math) overlapped with the
# on-device dense-node chains.
_USE_NP_GRAPH = os.environ.get("ATTGNN_NP_GRAPH", "1") == "1"

_jit_self = jax.jit(_f_self)
_jit_cross = jax.jit(_f_cross)
_jit_graph = jax.jit(_f_graph)
_jit_dense = jax.jit(_f_dense)
_jit_finish = jax.jit(_f_finish)


def kernel(desc0, desc1, kpts0, kpts1, params):
    import time as _time
    _t = {"self": 0.0, "pull": 0.0, "graph": 0.0, "dense_wait": 0.0,
          "finish": 0.0, "cross": 0.0}
    _tick = _time.time
    devs = jax.devices()[:B]
    g0_dev, g1_dev = devs[0], devs[4]

    params = jax.tree.map(np.asarray, params)
    put = jax.device_put

    # Replicate parameters (small) to every device once.
    lp_dev = [[put(jax.tree.map(jnp.asarray, lp), d) for lp in params["layers"]]
              for d in devs]
    dgmc_dev = [put({k: params["dgmc"][k] for k in params["dgmc"]}, d) for d in devs]
    fin_keys = {"m1": params["m1"], "m2": params["m2"],
                "n1g": params["n1g"], "n1b": params["n1b"]}
    fin_dev = [put(fin_keys, d) for d in devs]
    kp0_g = put(np.asarray(kpts0[0]), g0_dev)
    kp1_g = put(np.asarray(kpts1[0]), g1_dev)
    _np_kp0 = np.asarray(kpts0[0], dtype=np.float32)
    _np_kp1 = np.asarray(kpts1[0], dtype=np.float32)
    _np_dgmc = params["dgmc"]

    d0 = [put(np.asarray(desc0[b]), devs[b]) for b in range(B)]
    d1 = [put(np.asarray(desc1[b]), devs[b]) for b in range(B)]

    for li, name in enumerate(NAMES):
        if name == "cross":
            t0 = _tick()
            res = [_jit_cross(d0[b], d1[b], lp_dev[b][li]) for b in range(B)]
            d0 = [r[0] for r in res]
            d1 = [r[1] for r in res]
            jax.block_until_ready(d0)
            _t["cross"] += _tick() - t0
        else:
            t0 = _tick()
            res = [_jit_self(d0[b], d1[b], lp_dev[b][li]) for b in range(B)]
            # Pull the small idx / desc-slice outputs to host, assemble the
            # entangled graph-block inputs, dispatch side 0 -> dev0 and
            # side 1 -> dev4 while every device runs its dense chains.
            # Dispatch the per-batch dense chains first (async on all 8
            # devices), then run/issue the two entangled graph blocks.
            dense = [_jit_dense(d0[b], d1[b], dgmc_dev[b]) for b in range(B)]
            _t["self"] += _tick() - t0
            t0 = _tick()
            # 32 small device->host pulls; each is a tunnel roundtrip, so
            # overlap them across a thread pool.
            with ThreadPoolExecutor(16) as ex:
                flat = list(ex.map(np.asarray,
                                   [r[i] for r in res for i in (2, 3, 4, 5)]))
            idx0 = np.stack(flat[0::4])
            idx1 = np.stack(flat[1::4])
            ds0 = np.stack(flat[2::4])
            ds1 = np.stack(flat[3::4])
            _t["pull"] += _tick() - t0
            t0 = _tick()
            if _USE_NP_GRAPH:
                with ThreadPoolExecutor(2) as ex:
                    f0 = ex.submit(_np_graph, idx0, ds0, _np_kp0, _np_dgmc)
                    f1 = ex.submit(_np_graph, idx1, ds1, _np_kp1, _np_dgmc)
                    hg0_np = f0.result()
                    hg1_np = f1.result()
            else:
                hg0 = _jit_graph(put(idx0, g0_dev), put(ds0, g0_dev), kp0_g,
                                 dgmc_dev[0])
                hg1 = _jit_graph(put(idx1, g1_dev), put(ds1, g1_dev), kp1_g,
                                 dgmc_dev[4])
                hg0_np = np.asarray(hg0)
                hg1_np = np.asarray(hg1)
            _t["graph"] += _tick() - t0
            t0 = _tick()
            jax.block_until_ready(dense)
            _t["dense_wait"] += _tick() - t0
            t0 = _tick()
            new0, new1 = [], []
            for b in range(B):
                o = _jit_finish(res[b][0], res[b][1],
                                put(hg0_np, devs[b]), put(hg1_np, devs[b]),
                                dense[b][0], dense[b][1],
                                d0[b], d1[b],
                                put(np.int32(b), devs[b]), fin_dev[b])
                new0.append(o[0])
                new1.append(o[1])
            d0, d1 = new0, new1
            jax.block_until_ready(d0)
            _t["finish"] += _tick() - t0

    out0 = np.stack([np.asarray(x) for x in d0])
    out1 = np.stack([np.asarray(x) for x in d1])
    if os.environ.get("ATTGNN_TIMING", "0") == "1":
        print("phase times:", {k: round(v, 2) for k, v in _t.items()})
    return out0, out1


# revision 25
# speedup vs baseline: 2.6476x; 1.6254x over previous
"""AttentionalGNN (SuperGlue-style attention + DGMC spline message passing)
on 8 Trainium2 NeuronCores via the axon PJRT backend.

Sharding: data-parallel over batch B=8 (core b owns image pair b) for
attention / top-k / dense-node DGMC / MLPs. The reference's edge
construction uses per-batch top-k indices WITHOUT batch offsets, so all
batches' edges collide into flat nodes [0, 1024) = (n < 128, all b) —
that graph block is cross-batch entangled. It is computed once per side
(side 0 on device 0, side 1 on device 4), overlapped with the per-batch
dense-node chains on all devices, then broadcast for the final MLPs.

All math follows the reference exactly (fp32, erf-gelu, jax.lax.top_k).
"""

import os

_flags = os.environ.get("NEURON_CC_FLAGS", "")
if "--auto-cast" not in _flags:
    os.environ["NEURON_CC_FLAGS"] = (_flags + " --auto-cast=none").strip()

import numpy as np
import jax
import jax.numpy as jnp
from concurrent.futures import ThreadPoolExecutor

try:
    jax.config.update("jax_compilation_cache_dir", "/tmp/jax_neuron_cache")
    jax.config.update("jax_persistent_cache_min_entry_size_bytes", -1)
    jax.config.update("jax_persistent_cache_min_compile_time_secs", 0.0)
except Exception:
    pass

D_MODEL = 256
NHEAD = 4
HEAD_DIM = D_MODEL // NHEAD
NAMES = ["self", "cross", "self", "cross"]
KNN = 8
KSIZE = 5
GNN_DIM = 64
NUM_STEPS = 10
B = 8
N = 1024
NG = 1024          # graph-block flat nodes (ref flat index n*B+b < 1024)
ND = N - NG // B   # dense nodes per batch per side: n in [128, 1024)


def _layer_norm(x, g, b, eps=1e-5):
    m = x.mean(-1, keepdims=True)
    v = ((x - m) ** 2).mean(-1, keepdims=True)
    return (x - m) * jax.lax.rsqrt(v + eps) * g + b


def _linear(x, p):
    return x @ p["w"] + p["b"]


def _encoder(x_in, src_in, p):
    # x_in, src_in: [C, N] single batch. Returns delta [N, C], awm [N, N].
    x = _layer_norm(x_in.T, p["n0g"], p["n0b"])
    s = _layer_norm(src_in.T, p["n0g"], p["n0b"])
    q = _linear(x, p["q"]).reshape(N, NHEAD, HEAD_DIM)
    k = _linear(s, p["k"]).reshape(N, NHEAD, HEAD_DIM)
    v = _linear(s, p["v"]).reshape(N, NHEAD, HEAD_DIM)
    qk = jnp.einsum("nhd,mhd->nmh", q, k) / jnp.sqrt(jnp.float32(HEAD_DIM))
    A = jax.nn.softmax(qk, axis=1)                     # [N, M, H]
    msg = jnp.einsum("nmh,mhd->nhd", A, v).reshape(N, D_MODEL)
    msg = _layer_norm(_linear(msg, p["merge"]), p["n1g"], p["n1b"])
    msg = _linear(
        jax.nn.gelu(_linear(jnp.concatenate([x, msg], -1), p["mlp1"]),
                    approximate=False),
        p["mlp2"],
    )
    delta = _layer_norm(msg, p["n2g"], p["n2b"])
    awm = A.mean(axis=2)                               # [N, M]
    return delta, awm


def _f_self(desc0_b, desc1_b, lp):
    d0, awm0 = _encoder(desc0_b, desc0_b, lp)
    d1, awm1 = _encoder(desc1_b, desc1_b, lp)
    _, idx0 = jax.lax.top_k(awm0, KNN)
    _, idx1 = jax.lax.top_k(awm1, KNN)
    return d0, d1, idx0, idx1, desc0_b[:, :128], desc1_b[:, :128]


def _f_cross(desc0_b, desc1_b, lp):
    d0, _ = _encoder(desc0_b, desc1_b, lp)
    d1, _ = _encoder(desc1_b, desc0_b, lp)
    return desc0_b + d0.T, desc1_b + d1.T


def _spline_basis(attr):
    pos = attr * (KSIZE - 1)
    lo = jnp.clip(jnp.floor(pos), 0, KSIZE - 2).astype(jnp.int32)
    frac = pos - lo.astype(attr.dtype)
    corners = jnp.array([[0, 0], [0, 1], [1, 0], [1, 1]], jnp.int32)
    kidx = (lo[:, None, 0] + corners[None, :, 0]) * KSIZE + (
        lo[:, None, 1] + corners[None, :, 1])
    w0 = jnp.where(corners[None, :, 0] == 1, frac[:, None, 0], 1 - frac[:, None, 0])
    w1 = jnp.where(corners[None, :, 1] == 1, frac[:, None, 1], 1 - frac[:, None, 1])
    return kidx, w0 * w1


def _graph_conv(h, flat_idx, bw, cp):
    # h: [NG, 64]. flat_idx = src*25 + kidx, [E, 4]. Edge e = (b, n, j) has
    # dst n (ref layout: tile(repeat(arange(N), KNN), B)), so aggregation is
    # a dense reshape-sum — no scatter needed; deg == 64 everywhere.
    hkflat = (h @ cp["W"].transpose(1, 0, 2).reshape(GNN_DIM, -1))  # [NG, 25*64]
    hkflat = hkflat.reshape(NG, KSIZE * KSIZE, GNN_DIM).reshape(-1, GNN_DIM)
    gath = hkflat[flat_idx]                              # [E, 4, 64]
    msg = (bw[:, :, None] * gath).sum(1)                 # [E, 64]
    agg = msg.reshape(B, N, KNN, GNN_DIM).sum(axis=(0, 2)) / 64.0
    return agg + h @ cp["root"] + cp["b"]


def _dense_conv(h, cp):
    return h @ cp["root"] + cp["b"]


def _f_graph(idx_all, dslices, kp, dp):
    # idx_all: [B, N, KNN] i32 (per-batch top-k), dslices: [B, C, 128],
    # kp: [N, 2] (batch-0 keypoints — the reference's global-index quirk).
    # Returns final h over the NG entangled flat nodes.
    src = idx_all.reshape(-1)                            # [B*N*KNN] ref order
    base = jnp.repeat(jnp.arange(N), KNN)
    dst = jnp.tile(base, B)
    attr = kp[src] - kp[dst]
    amax = attr.max(0)
    amin = attr.min(0)
    attr = (attr - amin) / (amax - amin)
    kidx, bw = _spline_basis(attr)

    flat_idx = src[:, None] * (KSIZE * KSIZE) + kidx     # [E, 4]

    xg = jnp.transpose(dslices, (2, 0, 1)).reshape(NG, D_MODEL)  # row n*B+b
    h = jax.nn.relu(_linear(xg, dp["proj"]))

    def step(h, _):
        h1 = jax.nn.relu(_graph_conv(h, flat_idx, bw, dp["conv1"]))
        h2 = jax.nn.relu(_graph_conv(h1, flat_idx, bw, dp["conv2"]))
        h = h + _linear(jnp.concatenate([h, h1, h2], -1), dp["lin"])
        return h, None
    h, _ = jax.lax.scan(step, h, None, length=NUM_STEPS)
    return h


def _f_graph_dev(idx_all, dslices, kp, dp):
    # Device version of the entangled graph block. Builds the dense per-cell
    # adjacency A[k,n,m] = sum bw on device with one-hot einsums (no scatter,
    # no big host->device transfer), then runs the 20 convs as dense matmuls.
    KT = KSIZE * KSIZE
    src = idx_all.reshape(-1)                            # [E] ref order (b,n,j)
    base = jnp.repeat(jnp.arange(N), KNN)
    dst = jnp.tile(base, B)
    attr = kp[src] - kp[dst]
    amax = attr.max(0)
    amin = attr.min(0)
    attr = (attr - amin) / (amax - amin)
    kidx, bw = _spline_basis(attr)                       # [E,4], [E,4]

    # Per-edge cell weights: Wk[e, k] = sum_c bw[e,c] * (kidx[e,c] == k)
    wk = (jax.nn.one_hot(kidx, KT, dtype=jnp.float32)
          * bw[:, :, None]).sum(1)                       # [E, 25]
    # Group edges by dst n: slot t = (b, j), 64 per n.
    wk_n = wk.reshape(B, N, KNN, KT).transpose(1, 0, 2, 3).reshape(N, 64, KT)
    src_n = src.reshape(B, N, KNN).transpose(1, 0, 2).reshape(N, 64)
    oh_n = jax.nn.one_hot(src_n, N, dtype=jnp.float32)   # [N, 64, N]
    A = jnp.einsum("ntk,ntm->knm", wk_n, oh_n)           # [25, N, N]

    xg = jnp.transpose(dslices, (2, 0, 1)).reshape(NG, D_MODEL)
    h = jax.nn.relu(_linear(xg, dp["proj"]))

    def conv(x, cp):
        y = jnp.einsum("knm,mc->knc", A, x)              # [25, N, 64]
        agg = jnp.einsum("knc,kce->ne", y, cp["W"]) / 64.0
        return agg + x @ cp["root"] + cp["b"]

    def step(h, _):
        h1 = jax.nn.relu(conv(h, dp["conv1"]))
        h2 = jax.nn.relu(conv(h1, dp["conv2"]))
        h = h + _linear(jnp.concatenate([h, h1, h2], -1), dp["lin"])
        return h, None
    h, _ = jax.lax.scan(step, h, None, length=NUM_STEPS)
    return h


_jit_graph_dev = jax.jit(_f_graph_dev)


def _f_dense(desc0_b, desc1_b, dp):
    # Dense (non-entangled) flat nodes for this batch: n in [128, N).
    h = jax.nn.relu(_linear(
        jnp.concatenate([desc0_b[:, 128:], desc1_b[:, 128:]], 1).T,
        dp["proj"]))                                         # [2*896, 64]

    def step(h, _):
        h1 = jax.nn.relu(_dense_conv(h, dp["conv1"]))
        h2 = jax.nn.relu(_dense_conv(h1, dp["conv2"]))
        h = h + _linear(jnp.concatenate([h, h1, h2], -1), dp["lin"])
        return h, None
    h, _ = jax.lax.scan(step, h, None, length=NUM_STEPS)
    return h[:ND], h[ND:]


def _f_finish(delta0, delta1, hg0, hg1, hd0, hd1, desc0_b, desc1_b, b_idx, params):
    # Assemble d = [B,64,N] row for this batch: n<128 from the graph block
    # (flat n*B+b), n>=128 from the local dense chain.
    outs = []
    for delta, hg, hd, desc in ((delta0, hg0, hd0, desc0_b),
                                (delta1, hg1, hd1, desc1_b)):
        dg = jax.lax.dynamic_index_in_dim(
            hg.reshape(128, B, GNN_DIM), b_idx, axis=1, keepdims=False)
        d = jnp.concatenate([dg, hd], 0)                     # [N, 64]
        d = _layer_norm(d, params["n1g"], params["n1b"])
        cat = jnp.concatenate([delta, d], -1)                # [N, 320]
        m = _linear(jax.nn.gelu(_linear(cat, params["m1"]), approximate=False),
                    params["m2"])
        outs.append(desc + m.T)
    return outs[0], outs[1]


def _np_graph(idx_all, dslices, kp, dp):
    # Host fallback for the entangled graph block (exact same math, numpy).
    src = idx_all.reshape(-1).astype(np.int64)
    base = np.repeat(np.arange(N), KNN)
    dst = np.tile(base, B)
    attr = kp[src] - kp[dst]
    amax = attr.max(0)
    amin = attr.min(0)
    attr = (attr - amin) / (amax - amin)
    pos = attr * (KSIZE - 1)
    lo = np.clip(np.floor(pos), 0, KSIZE - 2).astype(np.int64)
    frac = (pos - lo).astype(np.float32)
    corners = np.array([[0, 0], [0, 1], [1, 0], [1, 1]], np.int64)
    kidx = (lo[:, None, 0] + corners[None, :, 0]) * KSIZE + (
        lo[:, None, 1] + corners[None, :, 1])
    w0 = np.where(corners[None, :, 0] == 1, frac[:, None, 0], 1 - frac[:, None, 0])
    w1 = np.where(corners[None, :, 1] == 1, frac[:, None, 1], 1 - frac[:, None, 1])
    bw = (w0 * w1).astype(np.float32)
    # Exact algebraic reorganization: agg[n] = (1/64) sum_k (A_k @ h) @ W_k
    # with A_k[n, m] = sum of bw over edge-corners (src=m, dst=n, cell=k).
    # A is fixed across all 20 convs of this layer-side; build it once with
    # bincount (C-speed) and turn every conv into pure BLAS — no gathers.
    KT = KSIZE * KSIZE
    idxA = (kidx.astype(np.int64) * (N * N)
            + dst[:, None].astype(np.int64) * N
            + src[:, None])                                  # [E, 4]
    A = np.bincount(idxA.ravel(), weights=bw.ravel().astype(np.float64),
                    minlength=KT * N * N)
    A2d = np.ascontiguousarray(A.reshape(KT * N, N).astype(np.float32))

    xg = np.transpose(dslices, (2, 0, 1)).reshape(NG, D_MODEL)
    h = np.maximum(xg @ dp["proj"]["w"] + dp["proj"]["b"], 0.0)

    def conv(x, cp):
        Y = (A2d @ x).reshape(KT, N, GNN_DIM)                # [25, N, 64]
        agg = Y[0] @ cp["W"][0]
        for k in range(1, KT):
            agg += Y[k] @ cp["W"][k]
        agg *= (1.0 / 64.0)
        return agg + x @ cp["root"] + cp["b"]

    for _ in range(NUM_STEPS):
        h1 = np.maximum(conv(h, dp["conv1"]), 0.0)
        h2 = np.maximum(conv(h1, dp["conv2"]), 0.0)
        h = h + np.concatenate([h, h1, h2], -1) @ dp["lin"]["w"] + dp["lin"]["b"]
    return h


# neuronx-cc densifies the [65536,4]-index gather into a petabyte-scale
# one-hot (NCC_EXSP001) with vector_dynamic_offsets DGE disabled, so the
# entangled graph block runs on host (exact same math) overlapped with the
# on-device dense-node chains.
_USE_NP_GRAPH = os.environ.get("ATTGNN_NP_GRAPH", "0") == "1"

_jit_self = jax.jit(_f_self)
_jit_cross = jax.jit(_f_cross)
_jit_graph = jax.jit(_f_graph)
_jit_dense = jax.jit(_f_dense)
_jit_finish = jax.jit(_f_finish)


def kernel(desc0, desc1, kpts0, kpts1, params):
    import time as _time
    _t = {"self": 0.0, "pull": 0.0, "graph": 0.0, "dense_wait": 0.0,
          "finish": 0.0, "cross": 0.0}
    _tick = _time.time
    devs = jax.devices()[:B]
    g0_dev, g1_dev = devs[0], devs[4]

    params = jax.tree.map(np.asarray, params)
    put = jax.device_put

    # Replicate parameters (small) to every device once.
    lp_dev = [[put(jax.tree.map(jnp.asarray, lp), d) for lp in params["layers"]]
              for d in devs]
    dgmc_dev = [put({k: params["dgmc"][k] for k in params["dgmc"]}, d) for d in devs]
    fin_keys = {"m1": params["m1"], "m2": params["m2"],
                "n1g": params["n1g"], "n1b": params["n1b"]}
    fin_dev = [put(fin_keys, d) for d in devs]
    kp0_g = put(np.asarray(kpts0[0]), g0_dev)
    kp1_g = put(np.asarray(kpts1[0]), g1_dev)
    _np_kp0 = np.asarray(kpts0[0], dtype=np.float32)
    _np_kp1 = np.asarray(kpts1[0], dtype=np.float32)
    _np_dgmc = params["dgmc"]

    d0 = [put(np.asarray(desc0[b]), devs[b]) for b in range(B)]
    d1 = [put(np.asarray(desc1[b]), devs[b]) for b in range(B)]

    for li, name in enumerate(NAMES):
        if name == "cross":
            t0 = _tick()
            res = [_jit_cross(d0[b], d1[b], lp_dev[b][li]) for b in range(B)]
            d0 = [r[0] for r in res]
            d1 = [r[1] for r in res]
            jax.block_until_ready(d0)
            _t["cross"] += _tick() - t0
        else:
            t0 = _tick()
            res = [_jit_self(d0[b], d1[b], lp_dev[b][li]) for b in range(B)]
            # Pull the small idx / desc-slice outputs to host, assemble the
            # entangled graph-block inputs, dispatch side 0 -> dev0 and
            # side 1 -> dev4 while every device runs its dense chains.
            # Dispatch the per-batch dense chains first (async on all 8
            # devices), then run/issue the two entangled graph blocks.
            dense = [_jit_dense(d0[b], d1[b], dgmc_dev[b]) for b in range(B)]
            _t["self"] += _tick() - t0
            t0 = _tick()
            # 32 small device->host pulls; each is a tunnel roundtrip, so
            # overlap them across a thread pool.
            with ThreadPoolExecutor(16) as ex:
                flat = list(ex.map(np.asarray,
                                   [r[i] for r in res for i in (2, 3, 4, 5)]))
            idx0 = np.stack(flat[0::4])
            idx1 = np.stack(flat[1::4])
            ds0 = np.stack(flat[2::4])
            ds1 = np.stack(flat[3::4])
            _t["pull"] += _tick() - t0
            t0 = _tick()
            if _USE_NP_GRAPH:
                with ThreadPoolExecutor(2) as ex:
                    f0 = ex.submit(_np_graph, idx0, ds0, _np_kp0, _np_dgmc)
                    f1 = ex.submit(_np_graph, idx1, ds1, _np_kp1, _np_dgmc)
                    hg0_np = f0.result()
                    hg1_np = f1.result()
            else:
                hg0 = _jit_graph_dev(put(idx0, g0_dev), put(ds0, g0_dev),
                                     kp0_g, dgmc_dev[0])
                hg1 = _jit_graph_dev(put(idx1, g1_dev), put(ds1, g1_dev),
                                     kp1_g, dgmc_dev[4])
                with ThreadPoolExecutor(2) as ex:
                    f0 = ex.submit(np.asarray, hg0)
                    f1 = ex.submit(np.asarray, hg1)
                    hg0_np = f0.result()
                    hg1_np = f1.result()
            _t["graph"] += _tick() - t0
            t0 = _tick()
            jax.block_until_ready(dense)
            _t["dense_wait"] += _tick() - t0
            t0 = _tick()
            new0, new1 = [], []
            for b in range(B):
                o = _jit_finish(res[b][0], res[b][1],
                                put(hg0_np, devs[b]), put(hg1_np, devs[b]),
                                dense[b][0], dense[b][1],
                                d0[b], d1[b],
                                put(np.int32(b), devs[b]), fin_dev[b])
                new0.append(o[0])
                new1.append(o[1])
            d0, d1 = new0, new1
            jax.block_until_ready(d0)
            _t["finish"] += _tick() - t0

    out0 = np.stack([np.asarray(x) for x in d0])
    out1 = np.stack([np.asarray(x) for x in d1])
    if os.environ.get("ATTGNN_TIMING", "0") == "1":
        print("phase times:", {k: round(v, 2) for k, v in _t.items()})
    return out0, out1
